# revision 2
# baseline (speedup 1.0000x reference)
"""Trainium2 Bass kernel v2 for the 2-layer GAT, tuned for the axon-emulated
TRN2 environment where per-instruction fixed costs dominate (matmul ~65us,
DVE ~30us+12.5ns/col, DMA ~150us, gpsimd gather ~2.9ms+0.55us/idx,
collective ~32ms flat).

Design (src-sharded nodes, feature-major everywhere, no dma_gather,
no one-hot matmuls, 4 collectives):
  - Node feature f=(h,o) lives at packed position (g,p): p = h*16 + o%16,
    g = o//16. fc1 output columns / fc2 input rows are host-permuted so
    matmul m-blocks emit packed blocks directly. head(p) = p//16 (g-indep).
  - Phase A computes fused features + z (packed SBUF table [128,7504,6]:
    [z g0..g3 | s_src_expanded | pad]) + expanded s_dst DRAM shards.
  - AllGather s_dst table [30000+, 128]. Edge aggregation: slots = edges
    padded per-dst to D = bucket(max_core_degree), dst order sigma1 =
    (shard-major, L2-dst-first, (D2cls, D1cls) sorted) -- host relabels
    nodes so all tables/partials are identity-ordered. Per window
    (~6k slots): one ap_gather from the SBUF z-table, per-class-run
    broadcast-add of s_dst, lrelu+exp, multiply, strided segmented
    reduce_sum -> H rows + den; DMA to partials; ReduceScatter.
  - MID: normalize+ELU, z2 = fc2(h') (packed), scores, z2 table + s2_dst
    shards; AllGather; layer-2 aggregation same machinery; ReduceScatter;
    normalize; host unpermutes rows.
"""
import os
import sys

sys.path.insert(0, "/opt/trn_rl_repo")

import numpy as np
import ml_dtypes

import concourse.bass as bass
import concourse.bacc as bacc
import concourse.tile as tile
import concourse.mybir as mybir
from concourse.bass_utils import run_bass_kernel_spmd

BF16 = ml_dtypes.bfloat16
NCORE = 8
F = 512
H = 8
OUT = 128
NS = 60000
SS = 7500
ND0 = 30000
ND1 = 8000
SH = 3750            # shard size over L1-dst space
CLS = (2, 4, 6, 8, 12, 16, 24, 32)
NIDX1 = int(os.environ.get("GAT_NIDX1", "6144"))
NIDX2 = int(os.environ.get("GAT_NIDX2", "8192"))
NDCAP = 1500         # max dsts per window (staging size)
WA = 512             # phase A node chunk


def _ceil(a, b):
    return -(-a // b)


def _bucket(v):
    for c in CLS:
        if v <= c:
            return c
    raise ValueError(v)


def _wrap_idx(idx):
    w = np.asarray(idx).reshape(-1, 16).T.astype(np.int16)
    return np.ascontiguousarray(np.tile(w, (8, 1)))


def _pack_perm():
    """packed col (g*128+p) -> original feature f."""
    pf = np.empty(512, np.int64)
    for g in range(4):
        for p in range(128):
            pf[g * 128 + p] = (p // 16) * 64 + g * 16 + (p % 16)
    return pf


def _windows(Dcls, nidx_max):
    """Cut the slot stream (dst-major, Dcls[q] slots for dst q) into windows.

    Pads each window with orphan zero-slots to ns % 16 == 0 (orphans belong
    to no dst: gathered as zero-node, in no class run). Returns
    (wins, S, base) with wins entries (so, ns_aligned, q0, nd, runs),
    runs=[(doff, ndr, D)], and base[q] = absolute slot offset of dst q."""
    nq = len(Dcls)
    wins = []
    base = np.zeros(nq + 1, np.int64)
    q = 0
    so = 0
    while q < nq:
        q0 = q
        ns = 0
        while q < nq and ns + Dcls[q] <= nidx_max and (q - q0) < NDCAP:
            base[q] = so + ns
            ns += Dcls[q]
            q += 1
        ns_al = _ceil(ns, 16) * 16
        runs = []
        r0 = q0
        for i in range(q0 + 1, q + 1):
            if i == q or Dcls[i] != Dcls[r0]:
                runs.append((r0 - q0, i - r0, int(Dcls[r0])))
                r0 = i
        wins.append((so, ns_al, q0, q - q0, tuple(runs)))
        so += ns_al
    return wins, so, base


def _slot_schedule(es, ed, sigpos_src, loc_src, sigpos_dst, ndst, zero_idx,
                   nidx_max):
    """Build per-core idx streams + window metadata.

    es/ed: original edge arrays. sigpos_src: orig src -> (core, local idx).
    sigpos_dst: orig dst -> sigma position (0..ndst).
    Returns (wins, S, idx[NCORE, S] int16, Dcls array)."""
    core = sigpos_src // SS if sigpos_src is None else None
    # caller passes loc_src = (core, local) tuple arrays
    ecore, eloc = loc_src
    sd = sigpos_dst[ed]
    deg = np.zeros((NCORE, ndst), np.int64)
    np.add.at(deg, (ecore, sd), 1)
    Dcls = np.array([_bucket(v) for v in deg.max(axis=0)], np.int64)
    wins, S = _windows(Dcls, nidx_max)
    base = np.zeros(ndst + 1, np.int64)
    np.cumsum(Dcls, out=base[1:])
    # window alignment padding is implicit: base matches stream exactly
    idx = np.full((NCORE, S), zero_idx, np.int64)
    for c in range(NCORE):
        m = ecore == c
        d = sd[m]
        s = eloc[m]
        o = np.argsort(d, kind="stable")
        d, s = d[o], s[o]
        starts = np.searchsorted(d, np.arange(ndst))
        rank = np.arange(len(d)) - starts[d]
        idx[c, base[d] + rank] = s
    return wins, S, idx, Dcls, base


def _prep(inputs):
    img = np.asarray(inputs["image_features"], np.float32)
    blk = np.asarray(inputs["block_features"], np.float32)
    W_img = np.asarray(inputs["W_img"], np.float32)
    W_blk = np.asarray(inputs["W_blk"], np.float32)
    Wv = np.asarray(inputs["Wv"], np.float32)
    bv = np.asarray(inputs["bv"], np.float32)
    We = np.asarray(inputs["We"], np.float32)
    be = np.asarray(inputs["be"], np.float32)
    fc1 = np.asarray(inputs["fc1"], np.float32)
    attn1 = np.asarray(inputs["attn1"], np.float32)
    fc2 = np.asarray(inputs["fc2"], np.float32)
    attn2 = np.asarray(inputs["attn2"], np.float32)
    e0s = np.asarray(inputs["edge0_src"], np.int64)
    e0d = np.asarray(inputs["edge0_dst"], np.int64)
    e1s = np.asarray(inputs["edge1_src"], np.int64)
    e1d = np.asarray(inputs["edge1_dst"], np.int64)
    assert int(inputs["n_dst0"]) == ND0 and int(inputs["n_dst1"]) == ND1
    IMG, BLKD = img.shape[1], blk.shape[1]
    KI, KB, MB = IMG // 128, BLKD // 128, F // 128
    O = F // H

    pf = _pack_perm()
    fc1T = np.ascontiguousarray(fc1.reshape(F, F).T)        # [fin, fout]
    fc1T_pk = np.ascontiguousarray(fc1T[:, pf]).astype(BF16)
    a_src = np.einsum("hof,ho->fh", fc1, attn1[:, :O])
    a_dst = np.einsum("hof,ho->fh", fc1, attn1[:, O:])
    acat = np.concatenate([a_src, a_dst], axis=1).astype(BF16)   # [F,16]
    fc2T = np.ascontiguousarray(fc2[0].T)                    # [F, OUT]
    fc2p = np.ascontiguousarray(fc2T[pf, :]).astype(BF16)    # [(g,p), OUT]
    # fc2p_sb layout [128, 4*OUT]: [p, g*OUT+j] = fc2T[pf[g*128+p], j]
    fc2p_sb = np.ascontiguousarray(
        fc2p.reshape(4, 128, OUT).transpose(1, 0, 2).reshape(128, 4 * OUT))
    a2cat = np.stack([attn2[0, :OUT], attn2[0, OUT:]], axis=1).astype(BF16)
    R16s = np.zeros((16, 128), np.float32)
    R16d = np.zeros((16, 128), np.float32)
    for p in range(128):
        R16s[p // 16, p] = 1.0
        R16d[8 + p // 16, p] = 1.0
    ones1 = np.ones((1, 128), np.float32).astype(BF16)
    biasv = np.ascontiguousarray(bv.reshape(MB, 128).T).astype(np.float32)
    biase = np.ascontiguousarray(be.reshape(MB, 128).T).astype(np.float32)

    # ---- degree/class computation (sigma-independent shard = orig//SH) ----
    shard0 = e0s // SS                     # L1 edge -> core (src owner)
    deg1 = np.zeros((NCORE, ND0), np.int64)
    np.add.at(deg1, (shard0, e0d), 1)
    D1 = np.array([_bucket(v) for v in deg1.max(axis=0)], np.int64)
    shard1 = e1s // SH                     # L2 edge -> core (RS1 shard owner)
    deg2 = np.zeros((NCORE, ND1), np.int64)
    np.add.at(deg2, (shard1, e1d), 1)
    D2 = np.array([_bucket(v) for v in deg2.max(axis=0)], np.int64)

    # ---- sigma1: per shard-half, L2-first, sorted (D2cls, D1cls, orig) ----
    order = []                              # orig ids, sigma position -> orig
    for sh in range(NCORE):
        nodes = np.arange(SH * sh, SH * (sh + 1))
        isl2 = nodes < ND1
        d2k = np.where(isl2, D2[np.minimum(nodes, ND1 - 1)], 0)
        key = (~isl2).astype(np.int64) * (1 << 40) + d2k * (1 << 20) + D1[nodes]
        o = np.argsort(key, kind="stable")
        order.append(nodes[o])
    sigma1 = np.concatenate(order)          # [ND0]
    sigpos = np.empty(ND0, np.int64)
    sigpos[sigma1] = np.arange(ND0)
    assert (sigma1[:ND1] < ND1).all() and sigpos[np.arange(ND1)].max() < ND1

    # per-core Phase A node order (orig ids)
    nodeorder = []
    for c in range(NCORE):
        if c < 4:
            nodeorder.append(np.concatenate([order[2 * c], order[2 * c + 1]]))
        else:
            nodeorder.append(np.arange(SS * c, SS * (c + 1)))
    # local new index of orig node within its core
    locidx = np.empty(NS, np.int64)
    for c in range(NCORE):
        locidx[nodeorder[c]] = np.arange(SS)

    # ---- L1 slot schedule (sigma order) ----
    D1s = D1[sigma1]
    wins1, S1, base1 = _windows(D1s, NIDX1)
    g1 = np.full((NCORE, S1), SS, np.int64)         # pad -> zero node 7500
    sd0 = sigpos[e0d]
    for c in range(NCORE):
        m = shard0 == c
        d, s = sd0[m], locidx[e0s[m]]
        o = np.argsort(d, kind="stable")
        d, s = d[o], s[o]
        starts = np.searchsorted(d, np.arange(ND0))
        rank = np.arange(len(d)) - starts[d]
        g1[c, base1[d] + rank] = s

    # ---- L2 slot schedule (sigma2 = sigma1 prefix, positions 0..ND1) ----
    sig2_of_orig = sigpos[np.arange(ND1)]           # orig L2 dst -> sigma2 pos
    D2s = np.empty(ND1, np.int64)
    D2s[sig2_of_orig] = D2                           # class by sigma2 position
    wins2, S2, base2 = _windows(D2s, NIDX2)
    g2 = np.full((NCORE, S2), SH, np.int64)          # pad -> zero node 3750
    sposs = sigpos[e1s]                              # src sigma1 position
    ecore2 = sposs // SH
    eloc2 = sposs % SH
    sd1 = sig2_of_orig[e1d]
    for c in range(NCORE):
        m = ecore2 == c
        d, s = sd1[m], eloc2[m]
        o = np.argsort(d, kind="stable")
        d, s = d[o], s[o]
        starts = np.searchsorted(d, np.arange(ND1))
        rank = np.arange(len(d)) - starts[d]
        g2[c, base2[d] + rank] = s

    shared = dict(fc1p=fc1T_pk, acat=acat, fc2p=fc2p_sb.astype(BF16),
                  a2cat=a2cat,
                  R16s=R16s.astype(BF16), R16d=R16d.astype(BF16),
                  ones1=ones1, biasv=biasv, biase=biase,
                  wimgT=np.ascontiguousarray(W_img.T).astype(BF16),
                  wblkT=np.ascontiguousarray(W_blk.T).astype(BF16),
                  wv=Wv.astype(BF16), we=We.astype(BF16),
                  tick=np.zeros((128, 1), np.float32))
    in_maps = []
    for c in range(NCORE):
        m = dict(shared)
        m["imgT"] = np.ascontiguousarray(img[nodeorder[c]].T).astype(BF16)
        m["blkT"] = np.ascontiguousarray(blk[nodeorder[c]].T).astype(BF16)
        m["g1idx"] = _wrap_idx(g1[c])
        m["g2idx"] = _wrap_idx(g2[c])
        in_maps.append(m)

    cfg = dict(IMG=IMG, BLK=BLKD, S1=int(S1), S2=int(S2),
               wins1=tuple(wins1), wins2=tuple(wins2))
    meta = dict(sigma1=sigma1)
    return cfg, in_maps, meta


# ---------------------------------------------------------------- device code

STOP_STAGE = int(os.environ.get("GAT_STOP_STAGE", "9"))
NT1 = SS + 4          # z table rows (incl zero node at SS)
NT2 = SH + 2          # z2 table rows (zero node at SH)
PR1 = 5               # partials1 row groups per partition: [H g0..3 | den]
PR2 = 2               # partials2: [h2 | den]


def _agg(nc, tc, ctx, *, ztab, ntab, dcols, zc, scol, sdx_dram, idx_dram,
         wins, S, partials, prg, name):
    """Edge aggregation: gather windows + weights + segmented reduce."""
    bf16 = mybir.dt.bfloat16
    f32 = mybir.dt.float32
    i16 = mybir.dt.int16
    TT = nc.vector.tensor_tensor
    MUL = mybir.AluOpType.mult
    ADD = mybir.AluOpType.add
    Exp = mybir.ActivationFunctionType.Exp
    gp = ctx.enter_context(tc.tile_pool(name=f"gp{name}", bufs=1))
    sp = ctx.enter_context(tc.tile_pool(name=f"sp{name}", bufs=1))
    for (so, ns, q0, nd, runs) in wins:
        idx_sb = sp.tile([128, ns // 16], i16, tag="idx")
        nc.sync.dma_start(idx_sb[:], idx_dram[:, so // 16:(so + ns) // 16])
        sdx_sb = sp.tile([128, nd], bf16, tag="sdx")
        nc.sync.dma_start(sdx_sb[:],
                          sdx_dram[q0:q0 + nd, :].rearrange("n p -> p n"))
        gb = gp.tile([128, ns * dcols], bf16, tag="gb")
        g3 = gb[:].rearrange("p (n d) -> p n d", d=dcols)
        nc.gpsimd.ap_gather(g3, ztab[:].rearrange("p (n d) -> p n d", d=dcols),
                            idx_sb[:, :], 128, ntab, dcols, ns)
        wb = gp.tile([128, ns], bf16, tag="wb")
        nc.vector.tensor_copy(wb[:], g3[:, :, scol])
        # s_dst add per class run
        off = 0
        for (doff, ndr, D) in runs:
            wrun = wb[:, off:off + ndr * D].rearrange("p (d j) -> p d j", j=D)
            TT(wrun, wrun,
               sdx_sb[:, doff:doff + ndr].unsqueeze(2).broadcast_to(
                   [128, ndr, D]), ADD)
            off += ndr * D
        nc.vector.scalar_tensor_tensor(wb[:], wb[:], 0.01, wb[:], MUL,
                                       mybir.AluOpType.max)
        nc.scalar.activation(wb[:], wb[:], Exp)
        TT(g3[:, :, 0:zc], g3[:, :, 0:zc],
           wb[:].unsqueeze(2).broadcast_to([128, ns, zc]), MUL)
        st = sp.tile([128, nd * prg], bf16, tag="st")
        st3 = st[:].rearrange("p (n g) -> p n g", g=prg)
        with nc.allow_low_precision(reason="<=32-term bf16 segment sums"):
            off = 0
            for (doff, ndr, D) in runs:
                grun = gb[:, off * dcols:(off + ndr * D) * dcols].rearrange(
                    "p (d j g) -> p d g j", j=D, g=dcols)
                nc.vector.reduce_sum(st3[:, doff:doff + ndr, 0:zc],
                                     grun[:, :, 0:zc, :], mybir.AxisListType.X)
                wrun = wb[:, off:off + ndr * D].rearrange("p (d j) -> p d j",
                                                          j=D)
                nc.vector.reduce_sum(st3[:, doff:doff + ndr, zc:zc + 1], wrun,
                                     mybir.AxisListType.X)
                off += ndr * D
        nc.sync.dma_start(
            partials[q0:q0 + nd, :].rearrange("d (p g) -> p d g", g=prg),
            st3[:, :, :])


def _build(cfg):
    stop = STOP_STAGE
    REPEAT = cfg.get("repeat", 1)
    bf16 = mybir.dt.bfloat16
    f32 = mybir.dt.float32
    i16 = mybir.dt.int16
    IMG, BLKD = cfg["IMG"], cfg["BLK"]
    S1, S2 = cfg["S1"], cfg["S2"]
    wins1, wins2 = cfg["wins1"], cfg["wins2"]
    KI, KB, MB = IMG // 128, BLKD // 128, F // 128
    DS1 = ND1 // NCORE

    nc = bacc.Bacc("TRN2", target_bir_lowering=False, debug=False,
                   num_devices=NCORE)

    def param(nm, shape, dt):
        return nc.declare_dram_parameter(nm, list(shape), dt, isOutput=False)

    imgT = param("imgT", [IMG, SS], bf16)
    blkT = param("blkT", [BLKD, SS], bf16)
    wimgT = param("wimgT", [IMG, F], bf16)
    wblkT = param("wblkT", [BLKD, F], bf16)
    wv = param("wv", [F, F], bf16)
    we = param("we", [F, F], bf16)
    fc1p = param("fc1p", [F, F], bf16)
    acat = param("acat", [F, 16], bf16)
    fc2p = param("fc2p", [128, 4 * OUT], bf16)
    a2cat = param("a2cat", [128, 2], bf16)
    R16s = param("R16s", [16, 128], bf16)
    R16d = param("R16d", [16, 128], bf16)
    ones1 = param("ones1", [1, 128], bf16)
    biasv = param("biasv", [128, MB], f32)
    biase = param("biase", [128, MB], f32)
    g1idxp = param("g1idx", [128, S1 // 16], i16)
    g2idxp = param("g2idx", [128, S2 // 16], i16)
    tick = param("tick", [128, 1], f32)
    out = nc.declare_dram_parameter("out", [DS1, OUT], f32, isOutput=True)
    tock = nc.declare_dram_parameter("tock", [128, 1], f32, isOutput=True)

    g1idx = nc.dram_tensor("g1idxd", [128, S1 // 16], i16)
    g2idx = nc.dram_tensor("g2idxd", [128, S2 // 16], i16)
    sdXsh1 = nc.dram_tensor("sdXsh1", [SS, 128], bf16)
    sdX1g = nc.dram_tensor("sdX1g", [NS, 128], bf16, addr_space="Shared")
    sdX1 = nc.dram_tensor("sdX1", [NS, 128], bf16)
    partials1 = nc.dram_tensor("partials1", [ND0, PR1 * 128], bf16)
    rs1 = nc.dram_tensor("rs1", [SH, PR1 * 128], bf16)
    sdXsh2 = nc.dram_tensor("sdXsh2", [SH, 128], bf16)
    sdX2g = nc.dram_tensor("sdX2g", [SH * NCORE, 128], bf16,
                           addr_space="Shared")
    sdX2 = nc.dram_tensor("sdX2", [SH * NCORE, 128], bf16)
    partials2 = nc.dram_tensor("partials2", [ND1, PR2 * 128], bf16)
    rs2 = nc.dram_tensor("rs2", [DS1, PR2 * 128], bf16)

    Sig = mybir.ActivationFunctionType.Sigmoid
    Exp = mybir.ActivationFunctionType.Exp
    TT = nc.vector.tensor_tensor
    MUL = mybir.AluOpType.mult
    ADD = mybir.AluOpType.add

    from contextlib import ExitStack
    with tile.TileContext(nc) as tc, ExitStack() as top:
        res = top.enter_context(tc.tile_pool(name="res", bufs=1))
        fc1p_sb = res.tile([128, MB * F], bf16)
        nc.sync.dma_start(fc1p_sb[:].rearrange("p (k m) -> p k m", k=MB),
                          fc1p[:, :].rearrange("(k p) m -> p k m", p=128))
        acat_sb = res.tile([128, MB * 16], bf16)
        nc.sync.dma_start(acat_sb[:].rearrange("p (k m) -> p k m", k=MB),
                          acat[:, :].rearrange("(k p) m -> p k m", p=128))
        fc2p_sb = res.tile([128, 4 * OUT], bf16)
        nc.sync.dma_start(fc2p_sb[:], fc2p[:, :])
        a2_sb = res.tile([128, 2], bf16)
        nc.sync.dma_start(a2_sb[:], a2cat[:, :])
        r16s_sb = res.tile([16, 128], bf16)
        nc.sync.dma_start(r16s_sb[:], R16s[:, :])
        r16d_sb = res.tile([16, 128], bf16)
        nc.sync.dma_start(r16d_sb[:], R16d[:, :])
        ones_sb = res.tile([1, 128], bf16)
        nc.sync.dma_start(ones_sb[:], ones1[:, :])
        bv_sb = res.tile([128, MB], f32)
        nc.sync.dma_start(bv_sb[:], biasv[:, :])
        be_sb = res.tile([128, MB], f32)
        nc.sync.dma_start(be_sb[:], biase[:, :])
        # idx tables to scratch DRAM (sliced per window later)
        nc.sync.dma_start(g1idx[:, :], g1idxp[:, :])
        nc.sync.dma_start(g2idx[:, :], g2idxp[:, :])

        tk = res.tile([128, 1], f32)
        nc.sync.dma_start(tk[:], tick[:, :])
        nc.sync.dma_start(tock[:, :], tk[:])
        if stop < 9:
            zo = res.tile([128, OUT], f32)
            nc.vector.memset(zo[:], 0.0)
            for t0 in range(0, DS1, 128):
                rows = min(128, DS1 - t0)
                nc.sync.dma_start(out[t0:t0 + rows, :], zo[:rows, :])

        for _rep in range(REPEAT):
          with ExitStack() as rep1:
            ztp = rep1.enter_context(tc.tile_pool(name=f"zt{_rep}", bufs=1))
            ztab = ztp.tile([128, NT1 * 6], bf16)
            zt3 = ztab[:].rearrange("p (n d) -> p n d", d=6)
            nc.vector.memset(ztab[:, SS * 6:NT1 * 6], 0.0)
            nc.vector.memset(zt3[:, SS:NT1, 4], -100000.0)

            # ---------------- Phase A ----------------
            with ExitStack() as pa:
              if True:
                wp = pa.enter_context(tc.tile_pool(name=f"aw{_rep}", bufs=1))
                rhsp = pa.enter_context(tc.tile_pool(name=f"ar{_rep}", bufs=2))
                sbp = pa.enter_context(tc.tile_pool(name=f"as{_rep}", bufs=1))
                psp = pa.enter_context(tc.tile_pool(name=f"ap{_rep}", bufs=2,
                                                    space="PSUM"))
                wimg_sb = wp.tile([128, KI * F], bf16)
                nc.sync.dma_start(
                    wimg_sb[:].rearrange("p (k m) -> p k m", k=KI),
                    wimgT[:, :].rearrange("(k p) m -> p k m", p=128))
                wblk_sb = wp.tile([128, KB * F], bf16)
                nc.sync.dma_start(
                    wblk_sb[:].rearrange("p (k m) -> p k m", k=KB),
                    wblkT[:, :].rearrange("(k p) m -> p k m", p=128))
                wv_sb = wp.tile([128, MB * F], bf16)
                nc.sync.dma_start(
                    wv_sb[:].rearrange("p (k m) -> p k m", k=MB),
                    wv[:, :].rearrange("(k p) m -> p k m", p=128))
                we_sb = wp.tile([128, MB * F], bf16)
                nc.sync.dma_start(
                    we_sb[:].rearrange("p (k m) -> p k m", k=MB),
                    we[:, :].rearrange("(k p) m -> p k m", p=128))
                for nt in range(_ceil(SS, WA)):
                    n0 = nt * WA
                    w = min(WA, SS - n0)
                    x_sb = rhsp.tile([128, KI * w], bf16, tag="x")
                    nc.sync.dma_start(
                        x_sb[:].rearrange("p (k n) -> p k n", k=KI),
                        imgT[:, n0:n0 + w].rearrange("(k p) n -> p k n",
                                                     p=128))
                    b_sb = rhsp.tile([128, KB * w], bf16, tag="b")
                    nc.sync.dma_start(
                        b_sb[:].rearrange("p (k n) -> p k n", k=KB),
                        blkT[:, n0:n0 + w].rearrange("(k p) n -> p k n",
                                                     p=128))

                    def mm(lhs_sb, rhs_sb, K, m, width):
                        ps = psp.tile([128, width], f32, tag="ps")
                        for k in range(K):
                            nc.tensor.matmul(
                                ps[:],
                                lhs_sb[:, k * F + m * 128:k * F + m * 128 + 128],
                                rhs_sb[:, k * width:(k + 1) * width],
                                start=(k == 0), stop=(k == K - 1))
                        return ps

                    fi_sb = sbp.tile([128, MB * w], bf16, tag="fi")
                    ti_sb = sbp.tile([128, MB * w], bf16, tag="ti")
                    av_sb = sbp.tile([128, MB * w], bf16, tag="av")
                    ae_sb = sbp.tile([128, MB * w], bf16, tag="ae")
                    for m in range(MB):
                        ps = mm(wimg_sb, x_sb, KI, m, w)
                        nc.vector.tensor_copy(fi_sb[:, m * w:(m + 1) * w],
                                              ps[:])
                    for m in range(MB):
                        ps = mm(wblk_sb, b_sb, KB, m, w)
                        nc.vector.tensor_copy(ti_sb[:, m * w:(m + 1) * w],
                                              ps[:])
                    for m in range(MB):
                        ps = mm(wv_sb, fi_sb, MB, m, w)
                        nc.scalar.activation(av_sb[:, m * w:(m + 1) * w],
                                             ps[:], Sig, bias=bv_sb[:, m:m + 1])
                    for m in range(MB):
                        ps = mm(we_sb, ti_sb, MB, m, w)
                        nc.scalar.activation(ae_sb[:, m * w:(m + 1) * w],
                                             ps[:], Sig, bias=be_sb[:, m:m + 1])
                    fu_sb = sbp.tile([128, MB * w], bf16, tag="fu")
                    TT(fu_sb[:], av_sb[:], fi_sb[:], MUL)
                    TT(ae_sb[:], ae_sb[:], ti_sb[:], MUL)
                    TT(fu_sb[:], fu_sb[:], ae_sb[:], ADD)
                    for g in range(MB):
                        ps = mm(fc1p_sb, fu_sb, MB, g, w)
                        nc.vector.tensor_copy(zt3[:, n0:n0 + w, g], ps[:])
                    pss = psp.tile([16, w], f32, tag="pss")
                    for k in range(MB):
                        nc.tensor.matmul(pss[:],
                                         acat_sb[:, k * 16:(k + 1) * 16],
                                         fu_sb[:, k * w:(k + 1) * w],
                                         start=(k == 0), stop=(k == MB - 1))
                    s_sb = sbp.tile([16, w], bf16, tag="s")
                    nc.vector.tensor_copy(s_sb[:], pss[:])
                    psx = psp.tile([128, w], f32, tag="psx")
                    nc.tensor.matmul(psx[:], r16s_sb[:], s_sb[:],
                                     start=True, stop=True)
                    nc.vector.tensor_copy(zt3[:, n0:n0 + w, 4], psx[:])
                    psd = psp.tile([128, w], f32, tag="psd")
                    nc.tensor.matmul(psd[:], r16d_sb[:], s_sb[:],
                                     start=True, stop=True)
                    sst = sbp.tile([128, w], bf16, tag="sst")
                    nc.vector.tensor_copy(sst[:], psd[:])
                    nc.sync.dma_start(
                        sdXsh1[n0:n0 + w, :].rearrange("n p -> p n"), sst[:])

            if stop >= 2:
                nc.gpsimd.collective_compute(
                    "AllGather", mybir.AluOpType.bypass,
                    replica_groups=[list(range(NCORE))],
                    ins=[sdXsh1[:, :]], outs=[sdX1g[:, :]])
                nc.sync.dma_start(sdX1[:, :], sdX1g[:, :])

            # ---------------- Layer-1 aggregation ----------------
            with ExitStack() as ag1:
                if stop >= 3:
                    _agg(nc, tc, ag1, ztab=ztab, ntab=NT1, dcols=6, zc=4,
                         scol=4, sdx_dram=sdX1, idx_dram=g1idx, wins=wins1,
                         S=S1, partials=partials1, prg=PR1, name=f"a{_rep}")

          if stop >= 4:
              nc.gpsimd.collective_compute(
                  "ReduceScatter", ADD, replica_groups=[list(range(NCORE))],
                  ins=[partials1[:, :]], outs=[rs1[:, :]])

          with ExitStack() as rep2:
            zt2p = rep2.enter_context(tc.tile_pool(name=f"z2{_rep}", bufs=1))
            ztab2 = zt2p.tile([128, NT2 * 4], bf16)
            z23 = ztab2[:].rearrange("p (n d) -> p n d", d=4)
            nc.vector.memset(ztab2[:, SH * 4:NT2 * 4], 0.0)
            nc.vector.memset(z23[:, SH:NT2, 1], -100000.0)

            # ---------------- MID: normalize + layer-2 tables ----------------
            with ExitStack() as p4:
              if stop >= 5:
                sbp = p4.enter_context(tc.tile_pool(name=f"m{_rep}", bufs=1))
                psp = p4.enter_context(tc.tile_pool(name=f"mp{_rep}", bufs=1,
                                                    space="PSUM"))
                for nt in range(_ceil(SH, WA)):
                    n0 = nt * WA
                    w = min(WA, SH - n0)
                    hs = sbp.tile([128, w * PR1], bf16, tag="hs")
                    h3 = hs[:].rearrange("p (n g) -> p n g", g=PR1)
                    nc.sync.dma_start(
                        h3, rs1[n0:n0 + w, :].rearrange("d (p g) -> p d g",
                                                        g=PR1))
                    rden = sbp.tile([128, w], f32, tag="rd")
                    nc.vector.reciprocal(rden[:], h3[:, :, 4])
                    hf = sbp.tile([128, 4 * w], f32, tag="hf")
                    hf3 = hf[:].rearrange("p (g n) -> p n g", n=w)
                    TT(hf3, h3[:, :, 0:4],
                       rden[:].unsqueeze(2).broadcast_to([128, w, 4]), MUL)
                    tmp = sbp.tile([128, 4 * w], f32, tag="tmp")
                    nc.vector.tensor_scalar_min(tmp[:], hf[:], 0.0)
                    nc.scalar.activation(tmp[:], tmp[:], Exp)
                    hb = sbp.tile([128, 4 * w], bf16, tag="hb")
                    nc.vector.scalar_tensor_tensor(hb[:], tmp[:], -1.0, hf[:],
                                                   ADD, mybir.AluOpType.max)
                    ps = psp.tile([128, w], f32, tag="z2")
                    for g in range(4):
                        nc.tensor.matmul(ps[:],
                                         fc2p_sb[:, g * OUT:(g + 1) * OUT],
                                         hb[:, g * w:(g + 1) * w],
                                         start=(g == 0), stop=(g == 3))
                    z2b = sbp.tile([128, w], bf16, tag="z2b")
                    nc.vector.tensor_copy(z2b[:], ps[:])
                    nc.vector.tensor_copy(z23[:, n0:n0 + w, 0], z2b[:])
                    ps2 = psp.tile([1, w], f32, tag="s2")
                    nc.tensor.matmul(ps2[:], a2_sb[:, 0:1], z2b[:],
                                     start=True, stop=True)
                    s2bs = sbp.tile([1, w], bf16, tag="s2bs")
                    nc.vector.tensor_copy(s2bs[:], ps2[:])
                    ps3 = psp.tile([1, w], f32, tag="s3")
                    nc.tensor.matmul(ps3[:], a2_sb[:, 1:2], z2b[:],
                                     start=True, stop=True)
                    s2bd = sbp.tile([1, w], bf16, tag="s2bd")
                    nc.vector.tensor_copy(s2bd[:], ps3[:])
                    pse = psp.tile([128, w], f32, tag="pse")
                    nc.tensor.matmul(pse[:], ones_sb[:], s2bs[:, :],
                                     start=True, stop=True)
                    nc.vector.tensor_copy(z23[:, n0:n0 + w, 1], pse[:])
                    psf = psp.tile([128, w], f32, tag="psf")
                    nc.tensor.matmul(psf[:], ones_sb[:], s2bd[:, :],
                                     start=True, stop=True)
                    sst = sbp.tile([128, w], bf16, tag="sst")
                    nc.vector.tensor_copy(sst[:], psf[:])
                    nc.sync.dma_start(
                        sdXsh2[n0:n0 + w, :].rearrange("n p -> p n"), sst[:])

            if stop >= 6:
                nc.gpsimd.collective_compute(
                    "AllGather", mybir.AluOpType.bypass,
                    replica_groups=[list(range(NCORE))],
                    ins=[sdXsh2[:, :]], outs=[sdX2g[:, :]])
                nc.sync.dma_start(sdX2[:, :], sdX2g[:, :])

            # ---------------- Layer-2 aggregation ----------------
            with ExitStack() as ag2:
                if stop >= 7:
                    _agg(nc, tc, ag2, ztab=ztab2, ntab=NT2, dcols=4, zc=1,
                         scol=1, sdx_dram=sdX2, idx_dram=g2idx, wins=wins2,
                         S=S2, partials=partials2, prg=PR2, name=f"b{_rep}")

          if stop >= 8:
              nc.gpsimd.collective_compute(
                  "ReduceScatter", ADD, replica_groups=[list(range(NCORE))],
                  ins=[partials2[:, :]], outs=[rs2[:, :]])

          # ---------------- final normalize ----------------
          with ExitStack() as p8:
            if stop >= 9:
                sbp = p8.enter_context(tc.tile_pool(name=f"f{_rep}", bufs=1))
                hs = sbp.tile([128, DS1 * PR2], bf16, tag="hs")
                h3 = hs[:].rearrange("p (n g) -> p n g", g=PR2)
                nc.sync.dma_start(
                    h3, rs2[:, :].rearrange("d (p g) -> p d g", g=PR2))
                rden = sbp.tile([128, DS1], f32, tag="rd")
                nc.vector.reciprocal(rden[:], h3[:, :, 1])
                of = sbp.tile([128, DS1], f32, tag="of")
                TT(of[:], h3[:, :, 0], rden[:], MUL)
                nc.sync.dma_start(out[:, :].rearrange("n p -> p n"), of[:])

    nc.compile()
    return nc


_CACHE = {}


def _get_nc(cfg):
    key = repr(sorted((k, repr(v)) for k, v in cfg.items()))
    if key not in _CACHE:
        _CACHE[key] = _build(cfg)
    return _CACHE[key]


def kernel(**inputs) -> np.ndarray:
    cfg, in_maps, meta = _prep(inputs)
    nc = _get_nc(cfg)
    res = run_bass_kernel_spmd(nc, in_maps, core_ids=list(range(NCORE)))
    sig = np.concatenate([res.results[c]["out"] for c in range(NCORE)], axis=0)
    full = np.empty_like(sig)
    full[meta["sigma1"][:ND1]] = sig
    return full


# revision 3
# speedup vs baseline: 1.0206x; 1.0206x over previous
"""Trainium2 Bass kernel v2 for the 2-layer GAT, tuned for the axon-emulated
TRN2 environment where per-instruction fixed costs dominate (matmul ~65us,
DVE ~30us+12.5ns/col, DMA ~150us, gpsimd gather ~2.9ms+0.55us/idx,
collective ~32ms flat).

Design (src-sharded nodes, feature-major everywhere, no dma_gather,
no one-hot matmuls, 4 collectives):
  - Node feature f=(h,o) lives at packed position (g,p): p = h*16 + o%16,
    g = o//16. fc1 output columns / fc2 input rows are host-permuted so
    matmul m-blocks emit packed blocks directly. head(p) = p//16 (g-indep).
  - Phase A computes fused features + z (packed SBUF table [128,7504,6]:
    [z g0..g3 | s_src_expanded | pad]) + expanded s_dst DRAM shards.
  - AllGather s_dst table [30000+, 128]. Edge aggregation: slots = edges
    padded per-dst to D = bucket(max_core_degree), dst order sigma1 =
    (shard-major, L2-dst-first, (D2cls, D1cls) sorted) -- host relabels
    nodes so all tables/partials are identity-ordered. Per window
    (~6k slots): one ap_gather from the SBUF z-table, per-class-run
    broadcast-add of s_dst, lrelu+exp, multiply, strided segmented
    reduce_sum -> H rows + den; DMA to partials; ReduceScatter.
  - MID: normalize+ELU, z2 = fc2(h') (packed), scores, z2 table + s2_dst
    shards; AllGather; layer-2 aggregation same machinery; ReduceScatter;
    normalize; host unpermutes rows.
"""
import os
import sys

sys.path.insert(0, "/opt/trn_rl_repo")

import numpy as np
import ml_dtypes

import concourse.bass as bass
import concourse.bacc as bacc
import concourse.tile as tile
import concourse.mybir as mybir
from concourse.bass_utils import run_bass_kernel_spmd

BF16 = ml_dtypes.bfloat16
NCORE = 8
F = 512
H = 8
OUT = 128
NS = 60000
SS = 7500
ND0 = 30000
ND1 = 8000
SH = 3750            # shard size over L1-dst space
CLS = (2, 4, 6, 8, 12, 16, 24, 32)
NIDX1 = int(os.environ.get("GAT_NIDX1", "6896"))
NIDX2 = int(os.environ.get("GAT_NIDX2", "8192"))
NDCAP1 = 1500        # max dsts per window (staging size)
NDCAP2 = 2200
WA = 512             # phase A node chunk


def _ceil(a, b):
    return -(-a // b)


def _bucket(v):
    for c in CLS:
        if v <= c:
            return c
    raise ValueError(v)


def _wrap_idx(idx):
    w = np.asarray(idx).reshape(-1, 16).T.astype(np.int16)
    return np.ascontiguousarray(np.tile(w, (8, 1)))


def _pack_perm():
    """packed col (g*128+p) -> original feature f."""
    pf = np.empty(512, np.int64)
    for g in range(4):
        for p in range(128):
            pf[g * 128 + p] = (p // 16) * 64 + g * 16 + (p % 16)
    return pf


def _windows(Dcls, nidx_max, ndcap):
    """Cut the slot stream (dst-major, Dcls[q] slots for dst q) into windows.

    Pads each window with orphan zero-slots to ns % 16 == 0 (orphans belong
    to no dst: gathered as zero-node, in no class run). Returns
    (wins, S, base) with wins entries (so, ns_aligned, q0, nd, runs),
    runs=[(doff, ndr, D)], and base[q] = absolute slot offset of dst q."""
    nq = len(Dcls)
    wins = []
    base = np.zeros(nq + 1, np.int64)
    q = 0
    so = 0
    while q < nq:
        q0 = q
        ns = 0
        while q < nq and ns + Dcls[q] <= nidx_max and (q - q0) < ndcap:
            base[q] = so + ns
            ns += Dcls[q]
            q += 1
        ns_al = _ceil(ns, 16) * 16
        runs = []
        r0 = q0
        for i in range(q0 + 1, q + 1):
            if i == q or Dcls[i] != Dcls[r0]:
                runs.append((r0 - q0, i - r0, int(Dcls[r0])))
                r0 = i
        wins.append((so, ns_al, q0, q - q0, tuple(runs)))
        so += ns_al
    return wins, so, base


def _slot_schedule(es, ed, sigpos_src, loc_src, sigpos_dst, ndst, zero_idx,
                   nidx_max):
    """Build per-core idx streams + window metadata.

    es/ed: original edge arrays. sigpos_src: orig src -> (core, local idx).
    sigpos_dst: orig dst -> sigma position (0..ndst).
    Returns (wins, S, idx[NCORE, S] int16, Dcls array)."""
    core = sigpos_src // SS if sigpos_src is None else None
    # caller passes loc_src = (core, local) tuple arrays
    ecore, eloc = loc_src
    sd = sigpos_dst[ed]
    deg = np.zeros((NCORE, ndst), np.int64)
    np.add.at(deg, (ecore, sd), 1)
    Dcls = np.array([_bucket(v) for v in deg.max(axis=0)], np.int64)
    wins, S = _windows(Dcls, nidx_max)
    base = np.zeros(ndst + 1, np.int64)
    np.cumsum(Dcls, out=base[1:])
    # window alignment padding is implicit: base matches stream exactly
    idx = np.full((NCORE, S), zero_idx, np.int64)
    for c in range(NCORE):
        m = ecore == c
        d = sd[m]
        s = eloc[m]
        o = np.argsort(d, kind="stable")
        d, s = d[o], s[o]
        starts = np.searchsorted(d, np.arange(ndst))
        rank = np.arange(len(d)) - starts[d]
        idx[c, base[d] + rank] = s
    return wins, S, idx, Dcls, base


def _prep(inputs):
    img = np.asarray(inputs["image_features"], np.float32)
    blk = np.asarray(inputs["block_features"], np.float32)
    W_img = np.asarray(inputs["W_img"], np.float32)
    W_blk = np.asarray(inputs["W_blk"], np.float32)
    Wv = np.asarray(inputs["Wv"], np.float32)
    bv = np.asarray(inputs["bv"], np.float32)
    We = np.asarray(inputs["We"], np.float32)
    be = np.asarray(inputs["be"], np.float32)
    fc1 = np.asarray(inputs["fc1"], np.float32)
    attn1 = np.asarray(inputs["attn1"], np.float32)
    fc2 = np.asarray(inputs["fc2"], np.float32)
    attn2 = np.asarray(inputs["attn2"], np.float32)
    e0s = np.asarray(inputs["edge0_src"], np.int64)
    e0d = np.asarray(inputs["edge0_dst"], np.int64)
    e1s = np.asarray(inputs["edge1_src"], np.int64)
    e1d = np.asarray(inputs["edge1_dst"], np.int64)
    assert int(inputs["n_dst0"]) == ND0 and int(inputs["n_dst1"]) == ND1
    IMG, BLKD = img.shape[1], blk.shape[1]
    KI, KB, MB = IMG // 128, BLKD // 128, F // 128
    O = F // H

    pf = _pack_perm()
    fc1T = np.ascontiguousarray(fc1.reshape(F, F).T)        # [fin, fout]
    fc1T_pk = np.ascontiguousarray(fc1T[:, pf]).astype(BF16)
    a_src = np.einsum("hof,ho->fh", fc1, attn1[:, :O])
    a_dst = np.einsum("hof,ho->fh", fc1, attn1[:, O:])
    acat = np.concatenate([a_src, a_dst], axis=1).astype(BF16)   # [F,16]
    fc2T = np.ascontiguousarray(fc2[0].T)                    # [F, OUT]
    fc2p = np.ascontiguousarray(fc2T[pf, :]).astype(BF16)    # [(g,p), OUT]
    # fc2p_sb layout [128, 4*OUT]: [p, g*OUT+j] = fc2T[pf[g*128+p], j]
    fc2p_sb = np.ascontiguousarray(
        fc2p.reshape(4, 128, OUT).transpose(1, 0, 2).reshape(128, 4 * OUT))
    a2cat = np.stack([attn2[0, :OUT], attn2[0, OUT:]], axis=1).astype(BF16)
    R16s = np.zeros((16, 128), np.float32)
    R16d = np.zeros((16, 128), np.float32)
    for p in range(128):
        R16s[p // 16, p] = 1.0
        R16d[8 + p // 16, p] = 1.0
    ones1 = np.ones((1, 128), np.float32).astype(BF16)
    biasv = np.ascontiguousarray(bv.reshape(MB, 128).T).astype(np.float32)
    biase = np.ascontiguousarray(be.reshape(MB, 128).T).astype(np.float32)

    # ---- degree/class computation (sigma-independent shard = orig//SH) ----
    shard0 = e0s // SS                     # L1 edge -> core (src owner)
    deg1 = np.zeros((NCORE, ND0), np.int64)
    np.add.at(deg1, (shard0, e0d), 1)
    D1 = np.array([_bucket(v) for v in deg1.max(axis=0)], np.int64)
    shard1 = e1s // SH                     # L2 edge -> core (RS1 shard owner)
    deg2 = np.zeros((NCORE, ND1), np.int64)
    np.add.at(deg2, (shard1, e1d), 1)
    D2 = np.array([_bucket(v) for v in deg2.max(axis=0)], np.int64)

    # ---- sigma1: per shard-half, L2-first, sorted (D2cls, D1cls, orig) ----
    order = []                              # orig ids, sigma position -> orig
    for sh in range(NCORE):
        nodes = np.arange(SH * sh, SH * (sh + 1))
        isl2 = nodes < ND1
        d2k = np.where(isl2, D2[np.minimum(nodes, ND1 - 1)], 0)
        key = (~isl2).astype(np.int64) * (1 << 40) + d2k * (1 << 20) + D1[nodes]
        o = np.argsort(key, kind="stable")
        order.append(nodes[o])
    sigma1 = np.concatenate(order)          # [ND0]
    sigpos = np.empty(ND0, np.int64)
    sigpos[sigma1] = np.arange(ND0)
    assert (sigma1[:ND1] < ND1).all() and sigpos[np.arange(ND1)].max() < ND1

    # per-core Phase A node order (orig ids)
    nodeorder = []
    for c in range(NCORE):
        if c < 4:
            nodeorder.append(np.concatenate([order[2 * c], order[2 * c + 1]]))
        else:
            nodeorder.append(np.arange(SS * c, SS * (c + 1)))
    # local new index of orig node within its core
    locidx = np.empty(NS, np.int64)
    for c in range(NCORE):
        locidx[nodeorder[c]] = np.arange(SS)

    # ---- L1 slot schedule (sigma order) ----
    D1s = D1[sigma1]
    wins1, S1, base1 = _windows(D1s, NIDX1, NDCAP1)
    g1 = np.full((NCORE, S1), SS, np.int64)         # pad -> zero node 7500
    sd0 = sigpos[e0d]
    for c in range(NCORE):
        m = shard0 == c
        d, s = sd0[m], locidx[e0s[m]]
        o = np.argsort(d, kind="stable")
        d, s = d[o], s[o]
        starts = np.searchsorted(d, np.arange(ND0))
        rank = np.arange(len(d)) - starts[d]
        g1[c, base1[d] + rank] = s

    # ---- L2 slot schedule (sigma2 = sigma1 prefix, positions 0..ND1) ----
    sig2_of_orig = sigpos[np.arange(ND1)]           # orig L2 dst -> sigma2 pos
    D2s = np.empty(ND1, np.int64)
    D2s[sig2_of_orig] = D2                           # class by sigma2 position
    wins2, S2, base2 = _windows(D2s, NIDX2, NDCAP2)
    g2 = np.full((NCORE, S2), SH, np.int64)          # pad -> zero node 3750
    sposs = sigpos[e1s]                              # src sigma1 position
    ecore2 = sposs // SH
    eloc2 = sposs % SH
    sd1 = sig2_of_orig[e1d]
    for c in range(NCORE):
        m = ecore2 == c
        d, s = sd1[m], eloc2[m]
        o = np.argsort(d, kind="stable")
        d, s = d[o], s[o]
        starts = np.searchsorted(d, np.arange(ND1))
        rank = np.arange(len(d)) - starts[d]
        g2[c, base2[d] + rank] = s

    shared = dict(fc1p=fc1T_pk, acat=acat, fc2p=fc2p_sb.astype(BF16),
                  a2cat=a2cat,
                  R16s=R16s.astype(BF16), R16d=R16d.astype(BF16),
                  ones1=ones1, biasv=biasv, biase=biase,
                  wimgT=np.ascontiguousarray(W_img.T).astype(BF16),
                  wblkT=np.ascontiguousarray(W_blk.T).astype(BF16),
                  wv=Wv.astype(BF16), we=We.astype(BF16),
                  tick=np.zeros((128, 1), np.float32))
    in_maps = []
    for c in range(NCORE):
        m = dict(shared)
        m["imgT"] = np.ascontiguousarray(img[nodeorder[c]].T).astype(BF16)
        m["blkT"] = np.ascontiguousarray(blk[nodeorder[c]].T).astype(BF16)
        m["g1idx"] = _wrap_idx(g1[c])
        m["g2idx"] = _wrap_idx(g2[c])
        in_maps.append(m)

    cfg = dict(IMG=IMG, BLK=BLKD, S1=int(S1), S2=int(S2),
               wins1=tuple(wins1), wins2=tuple(wins2))
    meta = dict(sigma1=sigma1)
    return cfg, in_maps, meta


# ---------------------------------------------------------------- device code

STOP_STAGE = int(os.environ.get("GAT_STOP_STAGE", "9"))
NT1 = SS + 4          # z table rows (incl zero node at SS)
NT2 = SH + 2          # z2 table rows (zero node at SH)
PR1 = 5               # partials1 row groups per partition: [H g0..3 | den]
PR2 = 2               # partials2: [h2 | den]


def _agg(nc, tc, ctx, *, ztab, ntab, dcols, zc, scol, sdx_dram, idx_dram,
         wins, S, partials, prg, name):
    """Edge aggregation: gather windows + weights + segmented reduce."""
    bf16 = mybir.dt.bfloat16
    f32 = mybir.dt.float32
    i16 = mybir.dt.int16
    TT = nc.vector.tensor_tensor
    MUL = mybir.AluOpType.mult
    ADD = mybir.AluOpType.add
    Exp = mybir.ActivationFunctionType.Exp
    gp = ctx.enter_context(tc.tile_pool(name=f"gp{name}", bufs=1))
    sp = ctx.enter_context(tc.tile_pool(name=f"sp{name}", bufs=1))
    for (so, ns, q0, nd, runs) in wins:
        idx_sb = sp.tile([128, ns // 16], i16, tag="idx")
        nc.sync.dma_start(idx_sb[:], idx_dram[:, so // 16:(so + ns) // 16])
        sdx_sb = sp.tile([128, nd], bf16, tag="sdx")
        nc.sync.dma_start(sdx_sb[:],
                          sdx_dram[q0:q0 + nd, :].rearrange("n p -> p n"))
        gb = gp.tile([128, ns * dcols], bf16, tag="gb")
        g3 = gb[:].rearrange("p (n d) -> p n d", d=dcols)
        nc.gpsimd.ap_gather(g3, ztab[:].rearrange("p (n d) -> p n d", d=dcols),
                            idx_sb[:, :], 128, ntab, dcols, ns)
        wb = gp.tile([128, ns], bf16, tag="wb")
        nc.vector.tensor_copy(wb[:], g3[:, :, scol])
        # s_dst add per class run
        off = 0
        for (doff, ndr, D) in runs:
            wrun = wb[:, off:off + ndr * D].rearrange("p (d j) -> p d j", j=D)
            TT(wrun, wrun,
               sdx_sb[:, doff:doff + ndr].unsqueeze(2).broadcast_to(
                   [128, ndr, D]), ADD)
            off += ndr * D
        nc.vector.scalar_tensor_tensor(wb[:], wb[:], 0.01, wb[:], MUL,
                                       mybir.AluOpType.max)
        nc.scalar.activation(wb[:], wb[:], Exp)
        TT(g3[:, :, 0:zc], g3[:, :, 0:zc],
           wb[:].unsqueeze(2).broadcast_to([128, ns, zc]), MUL)
        st = sp.tile([128, nd * prg], bf16, tag="st")
        st3 = st[:].rearrange("p (n g) -> p n g", g=prg)
        with nc.allow_low_precision(reason="<=32-term bf16 segment sums"):
            off = 0
            for (doff, ndr, D) in runs:
                grun = gb[:, off * dcols:(off + ndr * D) * dcols].rearrange(
                    "p (d j g) -> p d g j", j=D, g=dcols)
                nc.vector.reduce_sum(st3[:, doff:doff + ndr, 0:zc],
                                     grun[:, :, 0:zc, :], mybir.AxisListType.X)
                wrun = wb[:, off:off + ndr * D].rearrange("p (d j) -> p d j",
                                                          j=D)
                nc.vector.reduce_sum(st3[:, doff:doff + ndr, zc:zc + 1], wrun,
                                     mybir.AxisListType.X)
                off += ndr * D
        nc.sync.dma_start(
            partials[q0:q0 + nd, :].rearrange("d (p g) -> p d g", g=prg),
            st3[:, :, :])


def _build(cfg):
    stop = STOP_STAGE
    REPEAT = cfg.get("repeat", 1)
    bf16 = mybir.dt.bfloat16
    f32 = mybir.dt.float32
    i16 = mybir.dt.int16
    IMG, BLKD = cfg["IMG"], cfg["BLK"]
    S1, S2 = cfg["S1"], cfg["S2"]
    wins1, wins2 = cfg["wins1"], cfg["wins2"]
    KI, KB, MB = IMG // 128, BLKD // 128, F // 128
    DS1 = ND1 // NCORE

    nc = bacc.Bacc("TRN2", target_bir_lowering=False, debug=False,
                   num_devices=NCORE)

    def param(nm, shape, dt):
        return nc.declare_dram_parameter(nm, list(shape), dt, isOutput=False)

    imgT = param("imgT", [IMG, SS], bf16)
    blkT = param("blkT", [BLKD, SS], bf16)
    wimgT = param("wimgT", [IMG, F], bf16)
    wblkT = param("wblkT", [BLKD, F], bf16)
    wv = param("wv", [F, F], bf16)
    we = param("we", [F, F], bf16)
    fc1p = param("fc1p", [F, F], bf16)
    acat = param("acat", [F, 16], bf16)
    fc2p = param("fc2p", [128, 4 * OUT], bf16)
    a2cat = param("a2cat", [128, 2], bf16)
    R16s = param("R16s", [16, 128], bf16)
    R16d = param("R16d", [16, 128], bf16)
    ones1 = param("ones1", [1, 128], bf16)
    biasv = param("biasv", [128, MB], f32)
    biase = param("biase", [128, MB], f32)
    g1idxp = param("g1idx", [128, S1 // 16], i16)
    g2idxp = param("g2idx", [128, S2 // 16], i16)
    tick = param("tick", [128, 1], f32)
    out = nc.declare_dram_parameter("out", [DS1, OUT], f32, isOutput=True)
    tock = nc.declare_dram_parameter("tock", [128, 1], f32, isOutput=True)

    g1idx = nc.dram_tensor("g1idxd", [128, S1 // 16], i16)
    g2idx = nc.dram_tensor("g2idxd", [128, S2 // 16], i16)
    sdXsh1 = nc.dram_tensor("sdXsh1", [SS, 128], bf16)
    sdX1g = nc.dram_tensor("sdX1g", [NS, 128], bf16, addr_space="Shared")
    sdX1 = nc.dram_tensor("sdX1", [NS, 128], bf16)
    partials1 = nc.dram_tensor("partials1", [ND0, PR1 * 128], bf16)
    rs1 = nc.dram_tensor("rs1", [SH, PR1 * 128], bf16)
    sdXsh2 = nc.dram_tensor("sdXsh2", [SH, 128], bf16)
    sdX2g = nc.dram_tensor("sdX2g", [SH * NCORE, 128], bf16,
                           addr_space="Shared")
    sdX2 = nc.dram_tensor("sdX2", [SH * NCORE, 128], bf16)
    partials2 = nc.dram_tensor("partials2", [ND1, PR2 * 128], bf16)
    rs2 = nc.dram_tensor("rs2", [DS1, PR2 * 128], bf16)

    Sig = mybir.ActivationFunctionType.Sigmoid
    Exp = mybir.ActivationFunctionType.Exp
    TT = nc.vector.tensor_tensor
    MUL = mybir.AluOpType.mult
    ADD = mybir.AluOpType.add

    from contextlib import ExitStack
    with tile.TileContext(nc) as tc, ExitStack() as top:
        res = top.enter_context(tc.tile_pool(name="res", bufs=1))
        fc1p_sb = res.tile([128, MB * F], bf16)
        nc.sync.dma_start(fc1p_sb[:].rearrange("p (k m) -> p k m", k=MB),
                          fc1p[:, :].rearrange("(k p) m -> p k m", p=128))
        acat_sb = res.tile([128, MB * 16], bf16)
        nc.sync.dma_start(acat_sb[:].rearrange("p (k m) -> p k m", k=MB),
                          acat[:, :].rearrange("(k p) m -> p k m", p=128))
        fc2p_sb = res.tile([128, 4 * OUT], bf16)
        nc.sync.dma_start(fc2p_sb[:], fc2p[:, :])
        a2_sb = res.tile([128, 2], bf16)
        nc.sync.dma_start(a2_sb[:], a2cat[:, :])
        r16s_sb = res.tile([16, 128], bf16)
        nc.sync.dma_start(r16s_sb[:], R16s[:, :])
        r16d_sb = res.tile([16, 128], bf16)
        nc.sync.dma_start(r16d_sb[:], R16d[:, :])
        ones_sb = res.tile([1, 128], bf16)
        nc.sync.dma_start(ones_sb[:], ones1[:, :])
        bv_sb = res.tile([128, MB], f32)
        nc.sync.dma_start(bv_sb[:], biasv[:, :])
        be_sb = res.tile([128, MB], f32)
        nc.sync.dma_start(be_sb[:], biase[:, :])
        # idx tables to scratch DRAM (sliced per window later)
        nc.sync.dma_start(g1idx[:, :], g1idxp[:, :])
        nc.sync.dma_start(g2idx[:, :], g2idxp[:, :])

        tk = res.tile([128, 1], f32)
        nc.sync.dma_start(tk[:], tick[:, :])
        nc.sync.dma_start(tock[:, :], tk[:])
        if stop < 9:
            zo = res.tile([128, OUT], f32)
            nc.vector.memset(zo[:], 0.0)
            for t0 in range(0, DS1, 128):
                rows = min(128, DS1 - t0)
                nc.sync.dma_start(out[t0:t0 + rows, :], zo[:rows, :])

        for _rep in range(REPEAT):
          with ExitStack() as rep1:
            ztp = rep1.enter_context(tc.tile_pool(name=f"zt{_rep}", bufs=1))
            ztab = ztp.tile([128, NT1 * 6], bf16)
            zt3 = ztab[:].rearrange("p (n d) -> p n d", d=6)
            nc.vector.memset(ztab[:, SS * 6:NT1 * 6], 0.0)
            nc.vector.memset(zt3[:, SS:NT1, 4], -100000.0)

            # ---------------- Phase A ----------------
            with ExitStack() as pa:
              if True:
                wp = pa.enter_context(tc.tile_pool(name=f"aw{_rep}", bufs=1))
                rhsp = pa.enter_context(tc.tile_pool(name=f"ar{_rep}", bufs=2))
                sbp = pa.enter_context(tc.tile_pool(name=f"as{_rep}", bufs=1))
                psp = pa.enter_context(tc.tile_pool(name=f"ap{_rep}", bufs=2,
                                                    space="PSUM"))
                wimg_sb = wp.tile([128, KI * F], bf16)
                nc.sync.dma_start(
                    wimg_sb[:].rearrange("p (k m) -> p k m", k=KI),
                    wimgT[:, :].rearrange("(k p) m -> p k m", p=128))
                wblk_sb = wp.tile([128, KB * F], bf16)
                nc.sync.dma_start(
                    wblk_sb[:].rearrange("p (k m) -> p k m", k=KB),
                    wblkT[:, :].rearrange("(k p) m -> p k m", p=128))
                wv_sb = wp.tile([128, MB * F], bf16)
                nc.sync.dma_start(
                    wv_sb[:].rearrange("p (k m) -> p k m", k=MB),
                    wv[:, :].rearrange("(k p) m -> p k m", p=128))
                we_sb = wp.tile([128, MB * F], bf16)
                nc.sync.dma_start(
                    we_sb[:].rearrange("p (k m) -> p k m", k=MB),
                    we[:, :].rearrange("(k p) m -> p k m", p=128))
                for nt in range(_ceil(SS, WA)):
                    n0 = nt * WA
                    w = min(WA, SS - n0)
                    x_sb = rhsp.tile([128, KI * w], bf16, tag="x")
                    nc.sync.dma_start(
                        x_sb[:].rearrange("p (k n) -> p k n", k=KI),
                        imgT[:, n0:n0 + w].rearrange("(k p) n -> p k n",
                                                     p=128))
                    b_sb = rhsp.tile([128, KB * w], bf16, tag="b")
                    nc.sync.dma_start(
                        b_sb[:].rearrange("p (k n) -> p k n", k=KB),
                        blkT[:, n0:n0 + w].rearrange("(k p) n -> p k n",
                                                     p=128))

                    def mm(lhs_sb, rhs_sb, K, m, width):
                        ps = psp.tile([128, width], f32, tag="ps")
                        for k in range(K):
                            nc.tensor.matmul(
                                ps[:],
                                lhs_sb[:, k * F + m * 128:k * F + m * 128 + 128],
                                rhs_sb[:, k * width:(k + 1) * width],
                                start=(k == 0), stop=(k == K - 1))
                        return ps

                    fi_sb = sbp.tile([128, MB * w], bf16, tag="fi")
                    ti_sb = sbp.tile([128, MB * w], bf16, tag="ti")
                    av_sb = sbp.tile([128, MB * w], bf16, tag="av")
                    ae_sb = sbp.tile([128, MB * w], bf16, tag="ae")
                    for m in range(MB):
                        ps = mm(wimg_sb, x_sb, KI, m, w)
                        nc.vector.tensor_copy(fi_sb[:, m * w:(m + 1) * w],
                                              ps[:])
                    for m in range(MB):
                        ps = mm(wblk_sb, b_sb, KB, m, w)
                        nc.vector.tensor_copy(ti_sb[:, m * w:(m + 1) * w],
                                              ps[:])
                    for m in range(MB):
                        ps = mm(wv_sb, fi_sb, MB, m, w)
                        nc.scalar.activation(av_sb[:, m * w:(m + 1) * w],
                                             ps[:], Sig, bias=bv_sb[:, m:m + 1])
                    for m in range(MB):
                        ps = mm(we_sb, ti_sb, MB, m, w)
                        nc.scalar.activation(ae_sb[:, m * w:(m + 1) * w],
                                             ps[:], Sig, bias=be_sb[:, m:m + 1])
                    fu_sb = sbp.tile([128, MB * w], bf16, tag="fu")
                    TT(fu_sb[:], av_sb[:], fi_sb[:], MUL)
                    TT(ae_sb[:], ae_sb[:], ti_sb[:], MUL)
                    TT(fu_sb[:], fu_sb[:], ae_sb[:], ADD)
                    for g in range(MB):
                        ps = mm(fc1p_sb, fu_sb, MB, g, w)
                        nc.vector.tensor_copy(zt3[:, n0:n0 + w, g], ps[:])
                    pss = psp.tile([16, w], f32, tag="pss")
                    for k in range(MB):
                        nc.tensor.matmul(pss[:],
                                         acat_sb[:, k * 16:(k + 1) * 16],
                                         fu_sb[:, k * w:(k + 1) * w],
                                         start=(k == 0), stop=(k == MB - 1))
                    s_sb = sbp.tile([16, w], bf16, tag="s")
                    nc.vector.tensor_copy(s_sb[:], pss[:])
                    psx = psp.tile([128, w], f32, tag="psx")
                    nc.tensor.matmul(psx[:], r16s_sb[:], s_sb[:],
                                     start=True, stop=True)
                    nc.vector.tensor_copy(zt3[:, n0:n0 + w, 4], psx[:])
                    psd = psp.tile([128, w], f32, tag="psd")
                    nc.tensor.matmul(psd[:], r16d_sb[:], s_sb[:],
                                     start=True, stop=True)
                    sst = sbp.tile([128, w], bf16, tag="sst")
                    nc.vector.tensor_copy(sst[:], psd[:])
                    nc.sync.dma_start(
                        sdXsh1[n0:n0 + w, :].rearrange("n p -> p n"), sst[:])

            if stop >= 2:
                nc.gpsimd.collective_compute(
                    "AllGather", mybir.AluOpType.bypass,
                    replica_groups=[list(range(NCORE))],
                    ins=[sdXsh1[:, :]], outs=[sdX1g[:, :]])
                nc.sync.dma_start(sdX1[:, :], sdX1g[:, :])

            # ---------------- Layer-1 aggregation ----------------
            with ExitStack() as ag1:
                if stop >= 3:
                    _agg(nc, tc, ag1, ztab=ztab, ntab=NT1, dcols=6, zc=4,
                         scol=4, sdx_dram=sdX1, idx_dram=g1idx, wins=wins1,
                         S=S1, partials=partials1, prg=PR1, name=f"a{_rep}")

          if stop >= 4:
              nc.gpsimd.collective_compute(
                  "ReduceScatter", ADD, replica_groups=[list(range(NCORE))],
                  ins=[partials1[:, :]], outs=[rs1[:, :]])

          with ExitStack() as rep2:
            zt2p = rep2.enter_context(tc.tile_pool(name=f"z2{_rep}", bufs=1))
            ztab2 = zt2p.tile([128, NT2 * 4], bf16)
            z23 = ztab2[:].rearrange("p (n d) -> p n d", d=4)
            nc.vector.memset(ztab2[:, SH * 4:NT2 * 4], 0.0)
            nc.vector.memset(z23[:, SH:NT2, 1], -100000.0)

            # ---------------- MID: normalize + layer-2 tables ----------------
            with ExitStack() as p4:
              if stop >= 5:
                sbp = p4.enter_context(tc.tile_pool(name=f"m{_rep}", bufs=1))
                psp = p4.enter_context(tc.tile_pool(name=f"mp{_rep}", bufs=1,
                                                    space="PSUM"))
                for nt in range(_ceil(SH, WA)):
                    n0 = nt * WA
                    w = min(WA, SH - n0)
                    hs = sbp.tile([128, w * PR1], bf16, tag="hs")
                    h3 = hs[:].rearrange("p (n g) -> p n g", g=PR1)
                    nc.sync.dma_start(
                        h3, rs1[n0:n0 + w, :].rearrange("d (p g) -> p d g",
                                                        g=PR1))
                    rden = sbp.tile([128, w], f32, tag="rd")
                    nc.vector.reciprocal(rden[:], h3[:, :, 4])
                    hf = sbp.tile([128, 4 * w], f32, tag="hf")
                    hf3 = hf[:].rearrange("p (g n) -> p n g", n=w)
                    TT(hf3, h3[:, :, 0:4],
                       rden[:].unsqueeze(2).broadcast_to([128, w, 4]), MUL)
                    tmp = sbp.tile([128, 4 * w], f32, tag="tmp")
                    nc.vector.tensor_scalar_min(tmp[:], hf[:], 0.0)
                    nc.scalar.activation(tmp[:], tmp[:], Exp)
                    hb = sbp.tile([128, 4 * w], bf16, tag="hb")
                    nc.vector.scalar_tensor_tensor(hb[:], tmp[:], -1.0, hf[:],
                                                   ADD, mybir.AluOpType.max)
                    ps = psp.tile([128, w], f32, tag="z2")
                    for g in range(4):
                        nc.tensor.matmul(ps[:],
                                         fc2p_sb[:, g * OUT:(g + 1) * OUT],
                                         hb[:, g * w:(g + 1) * w],
                                         start=(g == 0), stop=(g == 3))
                    z2b = sbp.tile([128, w], bf16, tag="z2b")
                    nc.vector.tensor_copy(z2b[:], ps[:])
                    nc.vector.tensor_copy(z23[:, n0:n0 + w, 0], z2b[:])
                    ps2 = psp.tile([1, w], f32, tag="s2")
                    nc.tensor.matmul(ps2[:], a2_sb[:, 0:1], z2b[:],
                                     start=True, stop=True)
                    s2bs = sbp.tile([1, w], bf16, tag="s2bs")
                    nc.vector.tensor_copy(s2bs[:], ps2[:])
                    ps3 = psp.tile([1, w], f32, tag="s3")
                    nc.tensor.matmul(ps3[:], a2_sb[:, 1:2], z2b[:],
                                     start=True, stop=True)
                    s2bd = sbp.tile([1, w], bf16, tag="s2bd")
                    nc.vector.tensor_copy(s2bd[:], ps3[:])
                    pse = psp.tile([128, w], f32, tag="pse")
                    nc.tensor.matmul(pse[:], ones_sb[:], s2bs[:, :],
                                     start=True, stop=True)
                    nc.vector.tensor_copy(z23[:, n0:n0 + w, 1], pse[:])
                    psf = psp.tile([128, w], f32, tag="psf")
                    nc.tensor.matmul(psf[:], ones_sb[:], s2bd[:, :],
                                     start=True, stop=True)
                    sst = sbp.tile([128, w], bf16, tag="sst")
                    nc.vector.tensor_copy(sst[:], psf[:])
                    nc.sync.dma_start(
                        sdXsh2[n0:n0 + w, :].rearrange("n p -> p n"), sst[:])

            if stop >= 6:
                nc.gpsimd.collective_compute(
                    "AllGather", mybir.AluOpType.bypass,
                    replica_groups=[list(range(NCORE))],
                    ins=[sdXsh2[:, :]], outs=[sdX2g[:, :]])
                nc.sync.dma_start(sdX2[:, :], sdX2g[:, :])

            # ---------------- Layer-2 aggregation ----------------
            with ExitStack() as ag2:
                if stop >= 7:
                    _agg(nc, tc, ag2, ztab=ztab2, ntab=NT2, dcols=4, zc=1,
                         scol=1, sdx_dram=sdX2, idx_dram=g2idx, wins=wins2,
                         S=S2, partials=partials2, prg=PR2, name=f"b{_rep}")

          if stop >= 8:
              nc.gpsimd.collective_compute(
                  "ReduceScatter", ADD, replica_groups=[list(range(NCORE))],
                  ins=[partials2[:, :]], outs=[rs2[:, :]])

          # ---------------- final normalize ----------------
          with ExitStack() as p8:
            if stop >= 9:
                sbp = p8.enter_context(tc.tile_pool(name=f"f{_rep}", bufs=1))
                hs = sbp.tile([128, DS1 * PR2], bf16, tag="hs")
                h3 = hs[:].rearrange("p (n g) -> p n g", g=PR2)
                nc.sync.dma_start(
                    h3, rs2[:, :].rearrange("d (p g) -> p d g", g=PR2))
                rden = sbp.tile([128, DS1], f32, tag="rd")
                nc.vector.reciprocal(rden[:], h3[:, :, 1])
                of = sbp.tile([128, DS1], f32, tag="of")
                TT(of[:], h3[:, :, 0], rden[:], MUL)
                nc.sync.dma_start(out[:, :].rearrange("n p -> p n"), of[:])

    nc.compile()
    return nc


_CACHE = {}


def _get_nc(cfg):
    key = repr(sorted((k, repr(v)) for k, v in cfg.items()))
    if key not in _CACHE:
        _CACHE[key] = _build(cfg)
    return _CACHE[key]


def kernel(**inputs) -> np.ndarray:
    cfg, in_maps, meta = _prep(inputs)
    nc = _get_nc(cfg)
    res = run_bass_kernel_spmd(nc, in_maps, core_ids=list(range(NCORE)))
    sig = np.concatenate([res.results[c]["out"] for c in range(NCORE)], axis=0)
    full = np.empty_like(sig)
    full[meta["sigma1"][:ND1]] = sig
    return full
